# revision 3
# baseline (speedup 1.0000x reference)
"""Trainium2 Bass kernel for AudioQuantizer (VQ codebook lookup).

For x [N, 512], codebook [8192, 512], embedding [8192, 512]:
    dist[n,k] = ||x_n||^2 - 2 x_n.c_k + ||c_k||^2
    out[n]    = embedding[argmin_k dist[n,k]]

Sharding: data-parallel over N across 8 cores (codebook replicated).

Strategy (v2): the argmin only depends on v[n,k] = x_n.c_k - ||c_k||^2/2
(per-row constant ||x_n||^2 dropped).  The device computes v with a
SINGLE fp32r matmul pass (fp32r runs at full bf16 rate for moving dims
>= 256; measured precision ~2.6e-5 mean / ~1.5e-4 max abs error on this
data distribution -- roughly 13-bit-truncated products).  That is not
accurate enough to pick the argmax directly (the top-2 gap is often
smaller), so the device instead emits, per row, the MAXIMUM of v over
each 128-wide window of k (64 windows).  The host then selects every
window whose max is within MARGIN of the row's best window (MARGIN =
8e-4 >> any plausible fp32r error; ~1.006 windows/row on average) and
rescores those candidate windows exactly in fp32 with the reference's
rounding sequence, taking the true argmin.  The true argmin's window
can only be missed if a single fp32r entry erred by more than MARGIN
(~25 sigma of the measured error distribution), so in practice the
result matches the reference argmin exactly; a handful of flips would
still pass the 2e-2 rel-err gate by a wide margin.

Device work per core: 4096x8192x512 MACs in one fp32r pass (~440us of
tensor time) vs the previous 3-pass bf16-split scheme (~1.3ms).
Host work: ~4 GFLOP of sgemm rescoring + the embedding gather (the
gpsimd indirect-DMA gather is nonfunctional in this runtime).

The walrus build here encodes at most one sync-wait per instruction, so
after Tile scheduling we hoist excess waits onto standalone
EventSemaphore instructions (split_multi_waits).
"""

from contextlib import ExitStack

import numpy as np

import concourse.bass as bass
import concourse.mybir as mybir
import concourse.tile as tile
from concourse.bass_utils import run_bass_kernel_spmd
from concourse.masks import make_identity

F32 = mybir.dt.float32
F32R = mybir.dt.float32r

P = 128
KC = 512   # k-chunk: psum free dim per matmul group
WIN = 128  # candidate-window width (host rescore granularity)

N_CORES = 8
N_TOTAL = 32768
K_TOTAL = 8192
D = 512

MARGIN = 8e-4  # fp32r max-abs-error bound with ~5x headroom


def split_multi_waits(nc, max_waits=1):
    """Hoist excess sync-waits onto standalone EventSemaphore instructions.

    The walrus build here rejects instructions carrying more than one
    sync-wait ("Too many sync wait commands").  Tile attaches several.
    An EventSemaphore on the same engine queue immediately before the
    instruction is semantically equivalent (the queue stalls there).
    """
    n_new = 0
    for f in nc.m.functions:
        for bb in f.blocks:
            insts = list(bb.instructions)
            out = []
            for inst in insts:
                si = inst.sync_info
                waits = list(si.on_wait) if si is not None and si.on_wait else []
                if len(waits) > max_waits:
                    keep = waits[-max_waits:]
                    for i, w in enumerate(waits[:-max_waits]):
                        ev = mybir.InstEventSemaphore(
                            name=f"{inst.name}_hw{i}", ins=[], outs=[]
                        )
                        ev.engine = inst.engine
                        ev.sync_info = mybir.SyncInfo(on_wait=[w], on_update=[])
                        out.append(ev)
                        n_new += 1
                    inst.sync_info = mybir.SyncInfo(
                        on_wait=keep, on_update=list(si.on_update or [])
                    )
                out.append(inst)
            if len(out) != len(insts):
                bb.instructions = out
    return n_new


def build_kernel(n_shard=N_TOTAL // N_CORES, k_total=K_TOTAL, d=D):
    """Build the SPMD single-core program (same program runs on all cores)."""
    nc = bass.Bass("TRN2", target_bir_lowering=False, debug=False)

    n_tiles = n_shard // P          # 32
    n_chunks = k_total // KC        # 16
    kt_per_chunk = KC // P          # 4 codebook row-tiles per chunk
    d_chunks = d // P               # 4
    w_per_chunk = KC // WIN         # 4 windows per chunk
    n_windows = k_total // WIN      # 64

    x_ext = nc.dram_tensor("x", [n_shard, d], F32, kind="ExternalInput").ap()
    cb_ext = nc.dram_tensor("codebook", [k_total, d], F32, kind="ExternalInput").ap()
    cmax_ext = nc.dram_tensor(
        "cmax_out", [n_shard, n_windows], F32, kind="ExternalOutput"
    ).ap()

    with tile.TileContext(nc) as tc, ExitStack() as ctx:
        consts = ctx.enter_context(tc.tile_pool(name="consts", bufs=1))
        xT_pool = ctx.enter_context(tc.tile_pool(name="xT", bufs=1))
        cm_pool = ctx.enter_context(tc.tile_pool(name="cm", bufs=1))
        x_stage = ctx.enter_context(tc.tile_pool(name="x_stage", bufs=3))
        cb_stage = ctx.enter_context(tc.tile_pool(name="cb_stage", bufs=2))
        cbt_pool = ctx.enter_context(tc.tile_pool(name="cbt", bufs=2))
        sq_pool = ctx.enter_context(tc.tile_pool(name="sq", bufs=2))
        csq_pool = ctx.enter_context(tc.tile_pool(name="csq", bufs=2))
        mm_psum = ctx.enter_context(tc.tile_pool(name="mmps", bufs=5, space="PSUM"))
        tp_psum = ctx.enter_context(tc.tile_pool(name="tpps", bufs=2, space="PSUM"))
        bc_psum = ctx.enter_context(tc.tile_pool(name="bcps", bufs=1, space="PSUM"))

        identity = consts.tile([P, P], F32)
        make_identity(nc, identity[:])
        half_row = consts.tile([1, P], F32)
        nc.vector.memset(half_row[:], 0.5)

        # persistent per-core state: transposed x and window maxima
        xT = [
            xT_pool.tile([P, d], F32R, tag=f"xT{t}", name=f"xT{t}")
            for t in range(n_tiles)
        ]
        cmax = [
            cm_pool.tile([P, n_windows], F32, tag=f"cmax{t}", name=f"cmax{t}")
            for t in range(n_tiles)
        ]

        # ---- phase A: load + transpose x (xT[t][:, dc*P:(dc+1)*P] = x_tile^T) ----
        for t in range(n_tiles):
            xt = x_stage.tile([P, d], F32, name="xt")
            nc.sync.dma_start(xt[:], x_ext[t * P : (t + 1) * P, :])
            pst = tp_psum.tile([P, d], F32, tag="tp", name="pst")
            for dc in range(d_chunks):
                nc.tensor.transpose(
                    pst[:, dc * P : (dc + 1) * P], xt[:, dc * P : (dc + 1) * P],
                    identity[:],
                )
            nc.scalar.copy(xT[t][:], pst[:])

        # ---- phase B: per k-chunk: prep codebook, then sweep all n tiles ----
        for c in range(n_chunks):
            k0 = c * KC

            # stage 4 codebook row-tiles; accumulate c_sq
            cbs = []
            csq_cols = csq_pool.tile([P, kt_per_chunk], F32, tag="csqc", name="csqc")
            for kt in range(kt_per_chunk):
                cbt = cb_stage.tile([P, d], F32, tag=f"cb{kt}", name=f"cbt{kt}")
                nc.sync.dma_start(
                    cbt[:], cb_ext[k0 + kt * P : k0 + (kt + 1) * P, :]
                )
                cbs.append(cbt)
                sq = sq_pool.tile([P, d], F32, tag="sq", name="sq")
                nc.scalar.activation(
                    sq[:],
                    cbt[:],
                    mybir.ActivationFunctionType.Square,
                    accum_out=csq_cols[:, kt : kt + 1],
                )

            # transpose codebook chunk: cbT[dc] [P(d), KC(k)]
            cbT = []
            for dc in range(d_chunks):
                tpp = tp_psum.tile([P, KC], F32, tag="tp", name="tpp")
                for kt in range(kt_per_chunk):
                    nc.tensor.transpose(
                        tpp[:, kt * P : (kt + 1) * P],
                        cbs[kt][:, dc * P : (dc + 1) * P],
                        identity[:],
                    )
                cbt_sb = cbt_pool.tile([P, KC], F32R, tag=f"cbT{dc}", name=f"cbT{dc}")
                nc.scalar.copy(cbt_sb[:], tpp[:])
                cbT.append(cbt_sb)

            # c_sq column layout -> flat [1, KC] (k-major), then broadcast
            # 0.5*c_sq to all partitions via 0.5-row [P,1] x c_sq [1,KC] matmul
            csq_flat = csq_pool.tile([1, KC], F32, tag="csqf", name="csqf")
            for kt in range(kt_per_chunk):
                nc.sync.dma_start(
                    csq_flat[0:1, kt * P : (kt + 1) * P], csq_cols[:, kt : kt + 1]
                )
            bcp = bc_psum.tile([P, KC], F32, tag="bc", name="bcp")
            nc.tensor.matmul(bcp[:], half_row[:, :], csq_flat[0:1, :], start=True, stop=True)
            csq_b = csq_pool.tile([P, KC], F32, tag="csqb", name="csqb")
            nc.scalar.copy(csq_b[:], bcp[:])

            # main sweep: v = x.c - c_sq/2 in fp32r, windowed max into cmax
            for t in range(n_tiles):
                ps = mm_psum.tile([P, KC], F32, tag="mm", name="ps")
                for dc in range(d_chunks):
                    nc.tensor.matmul(
                        ps[:],
                        xT[t][:, dc * P : (dc + 1) * P],
                        cbT[dc][:],
                        start=(dc == 0),
                        stop=(dc == d_chunks - 1),
                    )
                nc.vector.tensor_sub(ps[:], ps[:], csq_b[:])
                nc.vector.tensor_reduce(
                    cmax[t][:, c * w_per_chunk : (c + 1) * w_per_chunk],
                    ps[:].rearrange("p (w i) -> p w i", i=WIN),
                    axis=mybir.AxisListType.X,
                    op=mybir.AluOpType.max,
                )

        # ---- phase C: window maxima to DRAM, row p of tile t -> row t*128+p ----
        for t in range(n_tiles):
            nc.sync.dma_start(cmax_ext[t * P : (t + 1) * P, :], cmax[t][:])

    return nc


_NC_CACHE = {}


def _get_nc():
    if "nc" not in _NC_CACHE:
        nc = build_kernel()
        split_multi_waits(nc)
        _NC_CACHE["nc"] = nc
    return _NC_CACHE["nc"]


def kernel(x, codebook, embedding, **run_kwargs):
    x = np.ascontiguousarray(np.asarray(x, dtype=np.float32))
    codebook = np.ascontiguousarray(np.asarray(codebook, dtype=np.float32))
    embedding = np.ascontiguousarray(np.asarray(embedding, dtype=np.float32))
    n = x.shape[0]
    n_shard = n // N_CORES
    nc = _get_nc()
    in_maps = [
        {"x": x[i * n_shard : (i + 1) * n_shard], "codebook": codebook}
        for i in range(N_CORES)
    ]
    res = run_bass_kernel_spmd(nc, in_maps, core_ids=list(range(N_CORES)), **run_kwargs)
    kernel.last_results = res
    cmax = np.concatenate(
        [res.results[i]["cmax_out"] for i in range(N_CORES)], axis=0
    )  # [N, 64]

    # ---- host: exact rescore of candidate windows ----
    n_windows = cmax.shape[1]
    gm = cmax.max(axis=1, keepdims=True)
    cand = cmax >= (gm - MARGIN)  # [N, 64] bool

    # reference rounding sequence: fl(fl(x_sq - 2*cross) + c_sq), fp32
    xsq = np.einsum("nd,nd->n", x.astype(np.float64), x.astype(np.float64))
    xsq = xsq.astype(np.float32)
    csq = np.einsum("kd,kd->k", codebook.astype(np.float64), codebook.astype(np.float64))
    csq = csq.astype(np.float32)

    best_val = np.full(n, np.inf, dtype=np.float32)
    best_idx = np.zeros(n, dtype=np.int64)
    for w in range(n_windows):
        rows = np.nonzero(cand[:, w])[0]
        if rows.size == 0:
            continue
        Cw = codebook[w * WIN : (w + 1) * WIN]
        cross = x[rows] @ Cw.T  # fp32 sgemm [nr, WIN]
        dist = (xsq[rows, None] - 2.0 * cross) + csq[None, w * WIN : (w + 1) * WIN]
        j = dist.argmin(axis=1)
        v = dist[np.arange(rows.size), j]
        upd = v < best_val[rows]  # ascending w: strict < keeps lowest k on ties
        ur = rows[upd]
        best_val[ur] = v[upd]
        best_idx[ur] = w * WIN + j[upd]

    return embedding[best_idx]


# revision 9
# speedup vs baseline: 1.2042x; 1.2042x over previous
"""Trainium2 Bass kernel for AudioQuantizer (VQ codebook lookup).

For x [N, 512], codebook [8192, 512], embedding [8192, 512]:
    dist[n,k] = ||x_n||^2 - 2 x_n.c_k + ||c_k||^2
    out[n]    = embedding[argmin_k dist[n,k]]

Sharding: data-parallel over N across 8 cores (codebook replicated).

Strategy (v2): the argmin only depends on v[n,k] = x_n.c_k - ||c_k||^2/2
(per-row constant ||x_n||^2 dropped).  The device computes v with a
SINGLE fp32r matmul pass (fp32r runs at full bf16 rate for moving dims
>= 256; measured precision ~2.6e-5 mean / ~1.5e-4 max abs error on this
data distribution -- roughly 13-bit-truncated products).  That is not
accurate enough to pick the argmax directly (the top-2 gap is often
smaller), so the device instead emits, per row, the MAXIMUM of v over
each 128-wide window of k (64 windows).  The host then selects every
window whose max is within MARGIN of the row's best window (MARGIN =
8e-4 >> any plausible fp32r error; ~1.006 windows/row on average) and
rescores those candidate windows exactly in fp32 with the reference's
rounding sequence, taking the true argmin.  The true argmin's window
can only be missed if a single fp32r entry erred by more than MARGIN
(~25 sigma of the measured error distribution), so in practice the
result matches the reference argmin exactly; a handful of flips would
still pass the 2e-2 rel-err gate by a wide margin.

Device work per core: 4096x8192x512 MACs in one fp32r pass (~440us of
tensor time) vs the previous 3-pass bf16-split scheme (~1.3ms).
Host work: ~4 GFLOP of sgemm rescoring + the embedding gather (the
gpsimd indirect-DMA gather is nonfunctional in this runtime).

The walrus build here encodes at most one sync-wait per instruction, so
after Tile scheduling we hoist excess waits onto standalone
EventSemaphore instructions (split_multi_waits).
"""

from contextlib import ExitStack

import numpy as np

import concourse.bass as bass
import concourse.mybir as mybir
import concourse.tile as tile
from concourse.bass_utils import run_bass_kernel_spmd
from concourse.masks import make_identity

F32 = mybir.dt.float32
F32R = mybir.dt.float32r

P = 128
KC = 512   # k-chunk: psum free dim per matmul group
WIN = 512  # candidate-window width (host rescore granularity)

N_CORES = 8
N_TOTAL = 32768
K_TOTAL = 8192
D = 512

MARGIN = 8e-4  # fp32r max-abs-error bound with ~5x headroom


def split_multi_waits(nc, max_waits=1):
    """Hoist excess sync-waits onto standalone EventSemaphore instructions.

    The walrus build here rejects instructions carrying more than one
    sync-wait ("Too many sync wait commands").  Tile attaches several.
    An EventSemaphore on the same engine queue immediately before the
    instruction is semantically equivalent (the queue stalls there).
    """
    n_new = 0
    for f in nc.m.functions:
        for bb in f.blocks:
            insts = list(bb.instructions)
            out = []
            for inst in insts:
                si = inst.sync_info
                waits = list(si.on_wait) if si is not None and si.on_wait else []
                if len(waits) > max_waits:
                    keep = waits[-max_waits:]
                    for i, w in enumerate(waits[:-max_waits]):
                        ev = mybir.InstEventSemaphore(
                            name=f"{inst.name}_hw{i}", ins=[], outs=[]
                        )
                        ev.engine = inst.engine
                        ev.sync_info = mybir.SyncInfo(on_wait=[w], on_update=[])
                        out.append(ev)
                        n_new += 1
                    inst.sync_info = mybir.SyncInfo(
                        on_wait=keep, on_update=list(si.on_update or [])
                    )
                out.append(inst)
            if len(out) != len(insts):
                bb.instructions = out
    return n_new


def build_kernel(n_shard=N_TOTAL // N_CORES, k_total=K_TOTAL, d=D):
    """Build the SPMD single-core program (same program runs on all cores)."""
    nc = bass.Bass("TRN2", target_bir_lowering=False, debug=False)

    n_tiles = n_shard // P          # 32
    n_chunks = k_total // KC        # 16
    kt_per_chunk = KC // P          # 4 codebook row-tiles per chunk
    d_chunks = d // P               # 4
    w_per_chunk = KC // WIN         # 4 windows per chunk
    n_windows = k_total // WIN      # 64

    x_ext = nc.dram_tensor("x", [n_shard, d], F32, kind="ExternalInput").ap()
    cb_ext = nc.dram_tensor("codebook", [k_total, d], F32, kind="ExternalInput").ap()
    cmax_ext = nc.dram_tensor(
        "cmax_out", [n_shard, n_windows], F32, kind="ExternalOutput"
    ).ap()

    with tile.TileContext(nc) as tc, ExitStack() as ctx:
        consts = ctx.enter_context(tc.tile_pool(name="consts", bufs=1))
        xT_pool = ctx.enter_context(tc.tile_pool(name="xT", bufs=1))
        cm_pool = ctx.enter_context(tc.tile_pool(name="cm", bufs=1))
        x_stage = ctx.enter_context(tc.tile_pool(name="x_stage", bufs=3))
        cb_stage = ctx.enter_context(tc.tile_pool(name="cb_stage", bufs=2))
        cbt_pool = ctx.enter_context(tc.tile_pool(name="cbt", bufs=2))
        mm_psum = ctx.enter_context(tc.tile_pool(name="mmps", bufs=6, space="PSUM"))
        tp_psum = ctx.enter_context(tc.tile_pool(name="tpps", bufs=2, space="PSUM"))

        identity = consts.tile([P, P], F32)
        make_identity(nc, identity[:])

        # persistent per-core state: transposed x and window maxima
        xT = [
            xT_pool.tile([P, d], F32R, tag=f"xT{t}", name=f"xT{t}")
            for t in range(n_tiles)
        ]
        cmax = [
            cm_pool.tile([P, n_windows], F32, tag=f"cmax{t}", name=f"cmax{t}")
            for t in range(n_tiles)
        ]

        # ---- phase A: load + transpose x (xT[t][:, dc*P:(dc+1)*P] = x_tile^T) ----
        for t in range(n_tiles):
            xt = x_stage.tile([P, d], F32, name="xt")
            nc.sync.dma_start(xt[:], x_ext[t * P : (t + 1) * P, :])
            pst = tp_psum.tile([P, d], F32, tag="tp", name="pst")
            for dc in range(d_chunks):
                nc.tensor.transpose(
                    pst[:, dc * P : (dc + 1) * P], xt[:, dc * P : (dc + 1) * P],
                    identity[:],
                )
            nc.scalar.copy(xT[t][:], pst[:])

        # ---- phase B: per k-chunk: prep codebook, then sweep all n tiles ----
        for c in range(n_chunks):
            k0 = c * KC

            # stage 4 codebook row-tiles
            cbs = []
            for kt in range(kt_per_chunk):
                cbt = cb_stage.tile([P, d], F32, tag=f"cb{kt}", name=f"cbt{kt}")
                nc.sync.dma_start(
                    cbt[:], cb_ext[k0 + kt * P : k0 + (kt + 1) * P, :]
                )
                cbs.append(cbt)

            # transpose codebook chunk: cbT[dc] [P(d), KC(k)]
            cbT = []
            for dc in range(d_chunks):
                tpp = tp_psum.tile([P, KC], F32, tag="tp", name="tpp")
                for kt in range(kt_per_chunk):
                    nc.tensor.transpose(
                        tpp[:, kt * P : (kt + 1) * P],
                        cbs[kt][:, dc * P : (dc + 1) * P],
                        identity[:],
                    )
                cbt_sb = cbt_pool.tile([P, KC], F32R, tag=f"cbT{dc}", name=f"cbT{dc}")
                nc.scalar.copy(cbt_sb[:], tpp[:])
                cbT.append(cbt_sb)

            # main sweep: cross = x.c in fp32r, per-chunk max into cmax
            for t in range(n_tiles):
                ps = mm_psum.tile([P, KC], F32, tag="mm", name="ps")
                for dc in range(d_chunks):
                    nc.tensor.matmul(
                        ps[:],
                        xT[t][:, dc * P : (dc + 1) * P],
                        cbT[dc][:],
                        start=(dc == 0),
                        stop=(dc == d_chunks - 1),
                    )
                nc.vector.tensor_reduce(
                    cmax[t][:, c : c + 1],
                    ps[:],
                    axis=mybir.AxisListType.X,
                    op=mybir.AluOpType.max,
                )

        # ---- phase C: window maxima to DRAM, row p of tile t -> row t*128+p ----
        for t in range(n_tiles):
            nc.sync.dma_start(cmax_ext[t * P : (t + 1) * P, :], cmax[t][:])

    return nc


_NC_CACHE = {}


def _get_nc():
    if "nc" not in _NC_CACHE:
        nc = build_kernel()
        split_multi_waits(nc)
        _NC_CACHE["nc"] = nc
    return _NC_CACHE["nc"]


def kernel(x, codebook, embedding, **run_kwargs):
    x = np.ascontiguousarray(np.asarray(x, dtype=np.float32))
    codebook = np.ascontiguousarray(np.asarray(codebook, dtype=np.float32))
    embedding = np.ascontiguousarray(np.asarray(embedding, dtype=np.float32))
    n = x.shape[0]
    n_shard = n // N_CORES

    # sort codebook rows by ||c||^2 so each device window has a tiny csq
    # spread; the device then only needs windowed maxima of raw cross=x.c
    csq64 = np.einsum(
        "kd,kd->k", codebook.astype(np.float64), codebook.astype(np.float64)
    )
    perm = np.argsort(csq64, kind="stable")
    cb_dev = np.ascontiguousarray(codebook[perm])

    nc = _get_nc()
    in_maps = [
        {"x": x[i * n_shard : (i + 1) * n_shard], "codebook": cb_dev}
        for i in range(N_CORES)
    ]
    res = run_bass_kernel_spmd(nc, in_maps, core_ids=list(range(N_CORES)), **run_kwargs)
    kernel.last_results = res
    cmax = np.concatenate(
        [res.results[i]["cmax_out"] for i in range(N_CORES)], axis=0
    )  # [N, n_windows] window maxima of cross (fp32r)

    n_windows = cmax.shape[1]
    csq_p = csq64[perm]  # ascending
    wmin = csq_p.reshape(n_windows, WIN).min(axis=1).astype(np.float32)  # [W]
    wmax = csq_p.reshape(n_windows, WIN).max(axis=1).astype(np.float32)  # [W]

    # v[n,k] = cross - csq/2.  Bounds per window from the device cross-max:
    #   ub_w >= max_{k in w} v   and   lb = max_w (achievable v in w)
    ub = (cmax - 0.5 * wmin[None, :]) + MARGIN
    lb = (cmax - 0.5 * wmax[None, :]) - MARGIN
    lb_best = lb.max(axis=1, keepdims=True)
    cand = ub >= lb_best  # [N, W]; the true argmin's window is always in here

    # exact rescore with the reference's fp32 rounding sequence and
    # first-occurrence (lowest ORIGINAL k) tie-breaking
    xsq = np.einsum("nd,nd->n", x.astype(np.float64), x.astype(np.float64))
    xsq = xsq.astype(np.float32)
    csq32 = csq64.astype(np.float32)

    BIGK = np.int64(1 << 40)
    best_val = np.full(n, np.inf, dtype=np.float32)
    best_k = np.full(n, BIGK, dtype=np.int64)
    for w in range(n_windows):
        rows = np.nonzero(cand[:, w])[0]
        if rows.size == 0:
            continue
        orig = perm[w * WIN : (w + 1) * WIN]  # original k of window entries
        Cw = cb_dev[w * WIN : (w + 1) * WIN]
        cross = x[rows] @ Cw.T  # fp32 sgemm [nr, WIN]
        dist = (xsq[rows, None] - 2.0 * cross) + csq32[None, orig]
        mv = dist.min(axis=1)
        # among ties at mv, the smallest original k
        mk = np.where(dist == mv[:, None], orig[None, :], BIGK).min(axis=1)
        better = (mv < best_val[rows]) | ((mv == best_val[rows]) & (mk < best_k[rows]))
        ur = rows[better]
        best_val[ur] = mv[better]
        best_k[ur] = mk[better]

    return embedding[best_k]
